# revision 1
# baseline (speedup 1.0000x reference)
"""Causal self-attention (B=1, T=4096, C=768, H=12) on 8 Trainium2 NeuronCores.

Sharding: tensor-parallel over heads. 16 head-slots across 8 cores (2 slots
per core); 12 real heads + 4 dummy slots with zeroed weights. Each core:
  1. transposes x -> x^T on the PE array (needed as matmul contraction layout)
  2. projects Q^T, K^T, V^T for its 2 head-slots (full T)
  3. runs causal flash-style attention fully on-chip in the transposed
     layout: S^T[k,q] = K^T.T @ Q^T per 128-wide k-block, P^T = exp(S^T/8)
     (scores are small enough that max-subtraction is unnecessary), causal
     masking via static triangular masks on the diagonal blocks only, and
     y^T accumulated in PSUM with an extra all-ones column in V providing
     the softmax denominator in row 64.
  4. normalizes y^T and computes a partial output projection with its
     128-row slice of w_proj.
The host sums the 8 partial [4096, 768] outputs -- no device collectives.

Causal load within a core is balanced by processing q-blocks in pairs
(i, 15-i) of 256 rows: each pair touches exactly 34 k-blocks.
"""

import sys

sys.path.insert(0, "/opt/trn_rl_repo")

import numpy as np

T = 4096
C = 768
H = 12
HD = 64
N_CORES = 8
SLOTS = 2
TS = 512  # t-slice for x load/transpose/projection
NTS = T // TS  # 8
QB = 256  # q-block rows
NQB = T // QB  # 16
KB = 128  # k-block rows
NKB = T // KB  # 32
NPAIR = NQB // 2  # 8 causal-balanced pairs (i, 15-i)

_CACHE = {}


def _paired_col(b256: int) -> int:
    """Column offset of 256-row q-block b256 in the paired SBUF layout.

    Pair p = min(b, 15-b) occupies cols [512p, 512p+512): side A (b < 8)
    at +0, side B (b >= 8) at +256.
    """
    p = min(b256, NQB - 1 - b256)
    side = 1 if b256 >= NQB // 2 else 0
    return 2 * QB * p + QB * side


def _build_nc():
    import concourse.bacc as bacc
    import concourse.tile as tile
    import concourse.mybir as mybir
    from concourse.masks import make_identity
    from contextlib import ExitStack

    F32 = mybir.dt.float32
    F32R = mybir.dt.float32r
    EXP = mybir.ActivationFunctionType.Exp

    nc = bacc.Bacc(
        "TRN2",
        target_bir_lowering=False,
        debug=False,
        enable_asserts=True,
        num_devices=N_CORES,
    )
    x_d = nc.dram_tensor("x", [T, C], F32R, kind="ExternalInput")
    wa_d = nc.dram_tensor("wa", [C, 3 * SLOTS * HD], F32R, kind="ExternalInput")
    wp_d = nc.dram_tensor("wp", [SLOTS * HD, C], F32R, kind="ExternalInput")
    out_d = nc.dram_tensor("out", [T, C], F32, kind="ExternalOutput")

    with ExitStack() as ctx:
        tc = ctx.enter_context(tile.TileContext(nc))
        singles = ctx.enter_context(tc.tile_pool(name="singles", bufs=1))
        xpool = ctx.enter_context(tc.tile_pool(name="xpool", bufs=8))
        xtpool = ctx.enter_context(tc.tile_pool(name="xtpool", bufs=8))
        ptpool = ctx.enter_context(tc.tile_pool(name="ptpool", bufs=8))
        rpool = ctx.enter_context(tc.tile_pool(name="rpool", bufs=4))
        opool = ctx.enter_context(tc.tile_pool(name="opool", bufs=4))
        ps = ctx.enter_context(tc.tile_pool(name="ps", bufs=2, space="PSUM"))
        ps_st = ctx.enter_context(tc.tile_pool(name="ps_st", bufs=2, space="PSUM"))
        ps_yt = ctx.enter_context(tc.tile_pool(name="ps_yt", bufs=2, space="PSUM"))

        # ---- persistent SBUF tensors ----
        qt = singles.tile([128, T], F32R)  # Q^T, paired column layout
        kt = singles.tile([128, T], F32R)  # K^T, natural column layout
        yt_all = singles.tile([128, T], F32R)  # normalized y^T, paired layout
        v1 = singles.tile([128, NKB, SLOTS, HD + 1], F32R)  # V blocks + ones col
        wa_sb = []
        for i in range(6):
            wa_c = singles.tile([128, 3 * SLOTS * HD], F32R, name=f"wa_c{i}")
            wa_sb.append(wa_c)
        wp_sb = singles.tile([SLOTS * HD, C], F32R)
        ident = singles.tile([128, 128], F32R)
        ones64 = singles.tile([1, HD], F32R)
        maskf = singles.tile([128, 3 * KB], F32)  # [:,128:384]=M0, [:,0:256]=M1

        ident_f32 = singles.tile([128, 128], F32)
        make_identity(nc, ident_f32)
        nc.vector.tensor_copy(out=ident, in_=ident_f32)
        ones_f32 = singles.tile([128, NKB * SLOTS], F32)
        nc.gpsimd.memset(ones_f32, 1.0)
        nc.vector.tensor_copy(out=ones64, in_=ones_f32[0:1, 0:HD])
        nc.vector.tensor_copy(
            out=v1[:, :, :, HD : HD + 1],
            in_=ones_f32.rearrange("p (a b) -> p a b", a=NKB).unsqueeze(3),
        )

        # maskf[k, c] = 1 if c >= k + 128 else 0
        nc.gpsimd.memset(maskf, 0.0)
        nc.gpsimd.affine_select(
            out=maskf,
            in_=maskf,
            compare_op=mybir.AluOpType.is_gt,
            fill=1.0,
            base=KB,
            channel_multiplier=1,
            pattern=[[-1, 3 * KB]],
        )

        for i in range(6):
            nc.gpsimd.dma_start(out=wa_sb[i], in_=wa_d.ap()[i * 128 : (i + 1) * 128, :])
        nc.gpsimd.dma_start(out=wp_sb, in_=wp_d.ap())

        # ---- phase A/B: x -> x^T -> Q^T/K^T/V per t-slice ----
        for ts in range(NTS):
            xs = []
            for tb in range(4):
                r0 = ts * TS + tb * 128
                xst = xpool.tile([128, C], F32R, name="xs", tag="xs")
                nc.sync.dma_start(out=xst, in_=x_d.ap()[r0 : r0 + 128, :])
                xs.append(xst)
            xts = []
            for ic in range(6):
                xtt = xtpool.tile([128, TS], F32R, name="xt", tag="xt")
                tps = ps.tile([128, TS], F32R, name="tps", tag="ps")
                for tb in range(4):
                    nc.tensor.transpose(
                        tps[:, tb * 128 : (tb + 1) * 128],
                        xs[tb][:, ic * 128 : (ic + 1) * 128],
                        ident,
                    )
                nc.vector.tensor_copy(out=xtt, in_=tps)
                xts.append(xtt)
            for p in range(3):
                pp = ps.tile([128, TS], F32, name="pp", tag="ps")
                for ic in range(6):
                    nc.tensor.matmul(
                        pp,
                        lhsT=wa_sb[ic][:, p * 128 : (p + 1) * 128],
                        rhs=xts[ic],
                        start=(ic == 0),
                        stop=(ic == 5),
                    )
                if p == 0:
                    for half in range(2):
                        col = _paired_col(2 * ts + half)
                        nc.vector.tensor_copy(
                            out=qt[:, col : col + QB],
                            in_=pp[:, half * QB : (half + 1) * QB],
                        )
                elif p == 1:
                    nc.vector.tensor_copy(out=kt[:, ts * TS : (ts + 1) * TS], in_=pp)
                else:
                    vt = rpool.tile([128, TS], F32R, name="vt", tag="vt", bufs=3)
                    nc.vector.tensor_copy(out=vt, in_=pp)
                    for sub in range(4):
                        kb = 4 * ts + sub
                        vps = ps.tile([128, 128], F32R, name="vps", tag="ps")
                        nc.tensor.transpose(
                            vps,
                            vt[:, sub * 128 : (sub + 1) * 128],
                            ident,
                        )
                        nc.vector.tensor_copy(
                            out=v1[:, kb, :, 0:HD],
                            in_=vps.rearrange("p (s d) -> p s d", s=SLOTS),
                        )

        # ---- phase C: attention, slots interleaved per pair; phase D
        # (partial projection) emitted as soon as a pair completes ----
        scale = 1.0 / float(np.sqrt(HD))

        def emit_proj(tb):
            b256, half = tb // 2, tb % 2
            col = _paired_col(b256) + 128 * half
            po = ps_st.tile([128, C], F32, name="po", tag="st")
            for c0, c1 in ((0, 512), (512, 768)):  # bank-aligned splits
                nc.tensor.matmul(
                    po[:, c0:c1],
                    lhsT=yt_all[:, col : col + 128],
                    rhs=wp_sb[:, c0:c1],
                    start=True,
                    stop=True,
                )
            osb = opool.tile([128, C], F32, name="osb", tag="osb")
            nc.vector.tensor_copy(out=osb, in_=po)
            nc.sync.dma_start(
                out=out_d.ap()[tb * 128 : (tb + 1) * 128, :], in_=osb
            )

        import collections
        work_q = collections.deque()  # deferred closures, drained between groups

        def emit_norm(ytsb, r0, r1, col):
            def go():
                r_sb = rpool.tile([1, 2 * QB], F32R, name="r_sb", tag="r_sb", bufs=8)
                with nc.allow_low_precision(reason="fp32r softmax denom"):
                    nc.vector.reciprocal(out=r_sb, in_=ytsb[HD : HD + 1, :])
                bc = ps.tile([HD, 2 * QB], F32, name="bc", tag="ps")
                nc.tensor.matmul(
                    bc,
                    lhsT=ones64,
                    rhs=r_sb,
                    start=True,
                    stop=True,
                )
                nc.vector.tensor_mul(
                    out=yt_all[r0:r1, col : col + 2 * QB],
                    in0=ytsb[0:HD, :],
                    in1=bc,
                )
            return go

        for i in reversed(range(NPAIR)):
            for s in range(SLOTS):
                r0, r1 = s * HD, (s + 1) * HD
                qcol = 2 * QB * i
                n_shared = 2 * i + 2  # k-blocks needed by side A (block i)
                n_total = NKB - 2 * i  # k-blocks needed by side B (block 15-i)
                diag_b0 = NKB - 2 - 2 * i  # first diagonal k-block of side B
                yt = ps_yt.tile([HD + 1, 2 * QB], F32, name="yt", tag="yt")
                # k-blocks in groups sharing one wide PSUM score tile:
                # shared region (A+B, q-width 512) pairs 2 k-blocks; solo
                # region (B only, q-width 256) packs 4. One exp per group.
                groups = [list(range(g, g + 2)) for g in range(0, n_shared, 2)]
                kb0 = n_shared
                while kb0 < n_total:
                    n = min(4, n_total - kb0)
                    groups.append(list(range(kb0, kb0 + n)))
                    kb0 += n
                def emit_s(grp):
                    shared = grp[0] < n_shared
                    w = 2 * QB if shared else QB
                    qoff = qcol if shared else qcol + QB
                    gw = w * len(grp)
                    st = ps_st.tile([128, 4 * QB], F32, name="st", tag="st")
                    for j, kb in enumerate(grp):
                        nc.tensor.matmul(
                            st[:, j * w : (j + 1) * w],
                            lhsT=kt[r0:r1, kb * KB : (kb + 1) * KB],
                            rhs=qt[r0:r1, qoff : qoff + w],
                            start=True,
                            stop=True,
                        )
                    pt = ptpool.tile([128, 4 * QB], F32R, name="pt", tag="pt")
                    nc.scalar.activation(
                        out=pt[:, 0:gw], in_=st[:, 0:gw], func=EXP, scale=scale
                    )
                    for j, kb in enumerate(grp):
                        pA = pt[:, j * w : j * w + QB]
                        if kb == 2 * i or kb == diag_b0:
                            nc.vector.tensor_mul(
                                out=pA, in0=pA, in1=maskf[:, KB : KB + QB]
                            )
                        if kb == 2 * i + 1 or kb == diag_b0 + 1:
                            nc.vector.tensor_mul(
                                out=pA, in0=pA, in1=maskf[:, 0:QB]
                            )
                    return pt, w

                def emit_pv(grp, pt, w):
                    # one PSUM accumulation group spans the whole pair:
                    # started once at kb==0 (full width), A columns simply
                    # stop being written after the shared region ends,
                    # stop flags on the final solo matmul
                    shared = grp[0] < n_shared
                    for j, kb in enumerate(grp):
                        vblk = v1[:, kb, s, :]
                        if shared:
                            nc.tensor.matmul(
                                yt,
                                lhsT=vblk,
                                rhs=pt[:, j * w : (j + 1) * w],
                                start=(kb == 0),
                                stop=False,
                                skip_group_check=True,
                            )
                        else:
                            nc.tensor.matmul(
                                yt[:, QB : 2 * QB],
                                lhsT=vblk,
                                rhs=pt[:, j * w : (j + 1) * w],
                                start=False,
                                stop=(kb == n_total - 1),
                                skip_group_check=True,
                            )

                pending = None
                for grp in groups:
                    cur = (grp, *emit_s(grp))
                    if pending is not None:
                        emit_pv(*pending)
                    pending = cur
                    if work_q:
                        work_q.popleft()()
                emit_pv(*pending)
                # free the yt PSUM slot immediately; queue the rest of
                # the normalization to drain between later matmul groups
                ytsb = rpool.tile([HD + 1, 2 * QB], F32, name="ytsb", tag="ytsb", bufs=6)
                nc.vector.tensor_copy(out=ytsb, in_=yt)
                work_q.append(emit_norm(ytsb, r0, r1, qcol))
            for tb in (2 * i, 2 * i + 1, NKB - 2 - 2 * i, NKB - 1 - 2 * i):
                work_q.append(lambda tb=tb: emit_proj(tb))
        while work_q:
            work_q.popleft()()


    nc.compile()
    return nc


def _get_nc():
    if "nc" not in _CACHE:
        _CACHE["nc"] = _build_nc()
    return _CACHE["nc"]


def _core_inputs(x, w_attn, w_proj):
    """Build per-core input dicts (head-slot weight slices)."""
    x = np.ascontiguousarray(x.reshape(T, C), dtype=np.float32)
    w_attn = np.asarray(w_attn, dtype=np.float32)
    w_proj = np.asarray(w_proj, dtype=np.float32)
    in_maps = []
    for c in range(N_CORES):
        heads = [c, 8 + c if c < 4 else None]
        wa = np.zeros((C, 3, SLOTS, HD), dtype=np.float32)
        wp = np.zeros((SLOTS * HD, C), dtype=np.float32)
        for s, h in enumerate(heads):
            if h is None:
                continue
            for p in range(3):
                wa[:, p, s, :] = w_attn[:, p * C + h * HD : p * C + (h + 1) * HD]
            wp[s * HD : (s + 1) * HD, :] = w_proj[h * HD : (h + 1) * HD, :]
        in_maps.append(
            {"x": x, "wa": np.ascontiguousarray(wa.reshape(C, 3 * SLOTS * HD)), "wp": wp}
        )
    return in_maps


def _get_runner():
    """Build the shard_map'd PJRT executable once and reuse it across calls.

    Mirrors bass2jax.run_bass_via_pjrt's multi-core path, but caches the
    jitted callable so repeat kernel() calls skip re-trace/re-compile.
    """
    if "runner" in _CACHE:
        return _CACHE["runner"]
    import jax
    import concourse.mybir as mybir
    from concourse import bass2jax
    from jax.experimental.shard_map import shard_map
    from jax.sharding import Mesh, PartitionSpec

    nc = _get_nc()
    bass2jax.install_neuronx_cc_hook()

    in_names, out_names, out_avals, zero_outs = [], [], [], []
    for alloc in nc.m.functions[0].allocations:
        if not isinstance(alloc, mybir.MemoryLocationSet):
            continue
        name = alloc.memorylocations[0].name
        if alloc.kind == "ExternalInput":
            if nc.partition_id_tensor and name == nc.partition_id_tensor.name:
                continue
            in_names.append(name)
        elif alloc.kind == "ExternalOutput":
            shape = tuple(alloc.tensor_shape)
            dtype = mybir.dt.np(alloc.dtype)
            out_names.append(name)
            out_avals.append(jax.core.ShapedArray(shape, dtype))
            zero_outs.append(np.zeros(shape, dtype))
    n_params = len(in_names)
    all_in_names = in_names + out_names
    if nc.partition_id_tensor:
        all_in_names = all_in_names + [nc.partition_id_tensor.name]

    def _body(*args):
        operands = list(args)
        if nc.partition_id_tensor:
            operands.append(bass2jax.partition_id_tensor())
        outs = bass2jax._bass_exec_p.bind(
            *operands,
            out_avals=tuple(out_avals),
            in_names=tuple(all_in_names),
            out_names=tuple(out_names),
            lowering_input_output_aliases=(),
            sim_require_finite=True,
            sim_require_nnan=True,
            nc=nc,
        )
        return tuple(outs)

    devices = jax.devices()[:N_CORES]
    mesh = Mesh(np.asarray(devices), ("core",))
    n_out = len(out_names)
    donate = tuple(range(n_params, n_params + n_out))
    sharded = jax.jit(
        shard_map(
            _body,
            mesh=mesh,
            in_specs=(PartitionSpec("core"),) * (n_params + n_out),
            out_specs=(PartitionSpec("core"),) * n_out,
            check_rep=False,
        ),
        donate_argnums=donate,
        keep_unused=True,
    )

    def run(in_maps):
        concat_in = [
            np.concatenate([in_maps[c][name] for c in range(N_CORES)], axis=0)
            for name in in_names
        ]
        concat_zeros = [
            np.zeros((N_CORES * z.shape[0], *z.shape[1:]), z.dtype)
            for z in zero_outs
        ]
        out_arrs = sharded(*concat_in, *concat_zeros)
        return [
            {
                name: np.asarray(out_arrs[i]).reshape(
                    N_CORES, *out_avals[i].shape
                )[c]
                for i, name in enumerate(out_names)
            }
            for c in range(N_CORES)
        ]

    _CACHE["runner"] = run
    return run


def kernel(x, w_attn, w_proj):
    run = _get_runner()
    in_maps = _core_inputs(np.asarray(x), np.asarray(w_attn), np.asarray(w_proj))
    results = run(in_maps)
    out = np.zeros((T, C), dtype=np.float32)
    for c in range(N_CORES):
        out += results[c]["out"]
    return out.reshape(1, T, C)



# revision 9
# speedup vs baseline: 1.3280x; 1.3280x over previous
"""Causal self-attention (B=1, T=4096, C=768, H=12) on 8 Trainium2 NeuronCores.

Sharding (uniform SPMD program, zero dummy work):
  - heads 0-7 live whole on cores 0-7 ("slot A", full softmax on device)
  - heads 8-11 are split between core pairs (c, c+4) by k-block PARITY:
    core c<4 handles even 128-row k-blocks, core c+4 the odd ones, each
    producing an UNNORMALIZED partial y plus the partial softmax
    denominator ("slot B"). The host adds the two partials and divides --
    exact because no max-subtraction is used (scores are small).
    The parity selection is pure data: the host packs x^T's k-blocks of
    the matching parity contiguously (xp), so both program variants are
    the same instruction stream.

Per core the device program:
  1. loads host-pretransposed x^T (bf16) -- no on-chip transposes of x
  2. projects Q^T/K^T (heads on partitions) and V in natural [t, d]
     layout (direct, no V transpose)
  3. causal attention per 256-row q-block: S^T = K^T.T Q^T per 128-wide
     k-block, exp on the scalar engine in 1536-wide batches, diagonal
     masks on gpsimd, then P^T.T V accumulated as y[q, d] with an extra
     ones-column in V giving the softmax denominator in column 64
  4. slot A: normalize y, transpose 128x64 tiles via the PE array, and
     project with this head's 64 rows of w_proj; slot B: ship raw y+denom
  The host sums the 8 partial projections and adds heads 8-11's
  contribution (a small [4096,256]x[256,768] matmul) in fp32.

All matmul inputs are bf16 (fp32 PSUM accumulation); the relative error
vs the fp32 reference stays ~1e-3, well inside the 2e-2 gate.
"""

import sys

sys.path.insert(0, "/opt/trn_rl_repo")

import numpy as np

T = 4096
C = 768
H = 12
HD = 64
N_CORES = 8
TS = 512  # t-slice for x load / projection
NTS = T // TS  # 8
QB = 256  # q-block rows
NQB = T // QB  # 16
KB = 128  # k-block rows
NKB = T // KB  # 32
GRP = 4  # k-blocks per score/exp group (4*256 = 1024 wide = 2 PSUM banks)

_CACHE = {}


def _groups(n):
    """Chunk k-block indices 0..n-1 into groups of GRP (ascending)."""
    return [list(range(g, min(g + GRP, n))) for g in range(0, n, GRP)]


def _build_nc():
    import concourse.bacc as bacc
    import concourse.tile as tile
    import concourse.mybir as mybir
    from concourse.masks import make_identity
    from contextlib import ExitStack
    import collections

    F32 = mybir.dt.float32
    BF16 = mybir.dt.bfloat16
    EXP = mybir.ActivationFunctionType.Exp
    GE = mybir.AluOpType.is_ge

    nc = bacc.Bacc(
        "TRN2",
        target_bir_lowering=False,
        debug=False,
        enable_asserts=True,
        num_devices=N_CORES,
    )
    # host-pretransposed x^T, full [C, T]
    xt_d = nc.dram_tensor("xt", [C, T], BF16, kind="ExternalInput")
    # x^T with only this core's parity of k-blocks, packed [C, T//2]
    xp_d = nc.dram_tensor("xp", [C, T // 2], BF16, kind="ExternalInput")
    wq_d = nc.dram_tensor("wq", [C, 2 * HD], BF16, kind="ExternalInput")
    wk_d = nc.dram_tensor("wk", [C, 2 * HD], BF16, kind="ExternalInput")
    wv_d = nc.dram_tensor("wv", [C, 2 * HD], BF16, kind="ExternalInput")
    wp_d = nc.dram_tensor("wp", [HD, C], BF16, kind="ExternalInput")
    mb_d = nc.dram_tensor("maskb", [KB, QB], BF16, kind="ExternalInput")
    out_d = nc.dram_tensor("out", [T, C], BF16, kind="ExternalOutput")
    yb_d = nc.dram_tensor("yb", [T, HD + 1], BF16, kind="ExternalOutput")

    scale = 1.0 / float(np.sqrt(HD))

    with ExitStack() as ctx:
        tc = ctx.enter_context(tile.TileContext(nc))
        singles = ctx.enter_context(tc.tile_pool(name="singles", bufs=1))
        ptpool = ctx.enter_context(tc.tile_pool(name="ptpool", bufs=4))
        opool = ctx.enter_context(tc.tile_pool(name="opool", bufs=2))
        rpool = ctx.enter_context(tc.tile_pool(name="rpool", bufs=4))
        ps_big = ctx.enter_context(tc.tile_pool(name="ps_big", bufs=2, space="PSUM"))
        ps_y = ctx.enter_context(tc.tile_pool(name="ps_y", bufs=2, space="PSUM"))
        ps_yt = ctx.enter_context(tc.tile_pool(name="ps_yt", bufs=2, space="PSUM"))

        # ---- persistent SBUF tensors ----
        xt = [singles.tile([128, T], BF16, name=f"xt{c}") for c in range(6)]
        xp = [singles.tile([128, T // 2], BF16, name=f"xp{c}") for c in range(6)]
        qt = singles.tile([128, T], BF16)  # rows 0:64 head A, 64:128 head B
        kt = singles.tile([128, T], BF16)  # B rows use cols 0:T//2 (packed)
        vA = singles.tile([128, NKB, HD + 1], BF16)
        vB = singles.tile([128, NKB // 2, HD + 1], BF16)
        y_sb = singles.tile([128, (T // KB) * HD], BF16)  # normalized y, head A
        wq_sb = singles.tile([128, 6, 2 * HD], BF16)
        wk_sb = singles.tile([128, 6, 2 * HD], BF16)
        wv_sb = singles.tile([128, 6, 2 * HD], BF16)
        wp_sb = singles.tile([HD, C], BF16)
        maskb = singles.tile([KB, QB], BF16)
        ident = singles.tile([128, 128], BF16)

        ident_f32 = singles.tile([128, 128], F32)
        make_identity(nc, ident_f32)
        nc.vector.tensor_copy(out=ident, in_=ident_f32)
        nc.gpsimd.memset(vA[:, :, HD : HD + 1], 1.0)
        nc.gpsimd.memset(vB[:, :, HD : HD + 1], 1.0)

        # ---- weight + mask DMAs (vector-engine queue; tiny) ----
        nc.scalar.dma_start(
            out=wq_sb, in_=wq_d.ap().rearrange("(c p) j -> p c j", p=128)
        )
        nc.scalar.dma_start(
            out=wk_sb, in_=wk_d.ap().rearrange("(c p) j -> p c j", p=128)
        )
        nc.scalar.dma_start(
            out=wv_sb, in_=wv_d.ap().rearrange("(c p) j -> p c j", p=128)
        )
        nc.scalar.dma_start(out=wp_sb, in_=wp_d.ap())
        nc.scalar.dma_start(out=maskb, in_=mb_d.ap())

        # ---- x^T input DMAs (sync-engine queue), finest slices first ----
        for c in range(6):
            nc.sync.dma_start(
                out=xt[c][:, 0:TS], in_=xt_d.ap()[128 * c : 128 * (c + 1), 0:TS]
            )
        for c in range(6):
            nc.sync.dma_start(
                out=xt[c][:, TS : 2 * TS],
                in_=xt_d.ap()[128 * c : 128 * (c + 1), TS : 2 * TS],
            )
        for c in range(6):
            nc.sync.dma_start(
                out=xp[c][:, 0:1024], in_=xp_d.ap()[128 * c : 128 * (c + 1), 0:1024]
            )
        for c in range(6):
            nc.sync.dma_start(
                out=xt[c][:, 1024:2048],
                in_=xt_d.ap()[128 * c : 128 * (c + 1), 1024:2048],
            )
        for c in range(6):
            nc.sync.dma_start(
                out=xt[c][:, 2048:3072],
                in_=xt_d.ap()[128 * c : 128 * (c + 1), 2048:3072],
            )
        for c in range(6):
            nc.sync.dma_start(
                out=xp[c][:, 1024:2048],
                in_=xp_d.ap()[128 * c : 128 * (c + 1), 1024:2048],
            )
        for c in range(6):
            nc.sync.dma_start(
                out=xt[c][:, 3072:4096],
                in_=xt_d.ap()[128 * c : 128 * (c + 1), 3072:4096],
            )

        # ---- emission helpers ----
        work_q = collections.deque()  # deferred closures to fill PE gaps

        def emit_main_proj(s):
            """q (A|B, full T), k_A, v_A for t-slice s."""
            big = ps_big.tile([128, 1024], F32, name="big", tag="big")
            c0, c1 = s * TS, (s + 1) * TS
            for c in range(6):
                nc.tensor.matmul(
                    big[:, 0:512],
                    lhsT=wq_sb[:, c, :],
                    rhs=xt[c][:, c0:c1],
                    start=(c == 0),
                    stop=(c == 5),
                )
            for c in range(6):
                nc.tensor.matmul(
                    big[0:64, 512:1024],
                    lhsT=wk_sb[:, c, 0:HD],
                    rhs=xt[c][:, c0:c1],
                    start=(c == 0),
                    stop=(c == 5),
                )
            bigv = ps_big.tile([128, 1024], F32, name="bigv", tag="big")
            for j in range(4):
                t0 = c0 + 128 * j
                for c in range(6):
                    nc.tensor.matmul(
                        bigv[:, 64 * j : 64 * (j + 1)],
                        lhsT=xt[c][:, t0 : t0 + 128],
                        rhs=wv_sb[:, c, 0:HD],
                        start=(c == 0),
                        stop=(c == 5),
                    )
            nc.vector.tensor_copy(out=qt[:, c0:c1], in_=big[:, 0:512])
            nc.vector.tensor_copy(out=kt[0:64, c0:c1], in_=big[0:64, 512:1024])
            nc.vector.tensor_copy(
                out=vA[:, 4 * s : 4 * s + 4, 0:HD],
                in_=bigv[:, 0:256].rearrange("p (j d) -> p j d", j=4),
            )

        def emit_packed_proj(p):
            """k_B, v_B for packed slice p (logical k-blocks 4p..4p+3)."""
            big = ps_big.tile([128, 1024], F32, name="bigp", tag="big")
            c0, c1 = p * TS, (p + 1) * TS
            for c in range(6):
                nc.tensor.matmul(
                    big[0:64, 0:512],
                    lhsT=wk_sb[:, c, HD : 2 * HD],
                    rhs=xp[c][:, c0:c1],
                    start=(c == 0),
                    stop=(c == 5),
                )
            for j in range(4):
                t0 = c0 + 128 * j
                for c in range(6):
                    nc.tensor.matmul(
                        big[:, 512 + 64 * j : 512 + 64 * (j + 1)],
                        lhsT=xp[c][:, t0 : t0 + 128],
                        rhs=wv_sb[:, c, HD : 2 * HD],
                        start=(c == 0),
                        stop=(c == 5),
                    )
            nc.vector.tensor_copy(out=kt[64:128, c0:c1], in_=big[0:64, 0:512])
            nc.vector.tensor_copy(
                out=vB[:, 4 * p : 4 * p + 4, 0:HD],
                in_=big[:, 512:768].rearrange("p (j d) -> p j d", j=4),
            )

        # attention tasks: one per (block b, slot, k-group)
        def emit_scores(t):
            b, slot, kbs, _, _ = t
            r0, r1 = (0, 64) if slot == 0 else (64, 128)
            gw = 256 * len(kbs)
            st = ps_big.tile([128, 1024], F32, name="st", tag="big")
            for j, kb in enumerate(kbs):
                nc.tensor.matmul(
                    st[:, 256 * j : 256 * (j + 1)],
                    lhsT=kt[r0:r1, 128 * kb : 128 * (kb + 1)],
                    rhs=qt[r0:r1, QB * b : QB * (b + 1)],
                    start=True,
                    stop=True,
                )
            pt = ptpool.tile([128, 1024], BF16, name="pt", tag="pt")
            nc.scalar.activation(out=pt[:, 0:gw], in_=st[:, 0:gw], func=EXP, scale=scale)
            if t[3]:  # last group: diagonal causal masks
                nd = len(kbs)
                if slot == 0:
                    # phys diag blocks 2b (keep q>=k: col>=p) and 2b+1
                    # (keep col >= 128+p), in place on gpsimd
                    nc.gpsimd.affine_select(
                        out=pt[:, 256 * (nd - 2) : 256 * (nd - 1)],
                        in_=pt[:, 256 * (nd - 2) : 256 * (nd - 1)],
                        compare_op=GE,
                        fill=0.0,
                        base=0,
                        channel_multiplier=-1,
                        pattern=[[1, QB]],
                    )
                    nc.gpsimd.affine_select(
                        out=pt[:, 256 * (nd - 1) : 256 * nd],
                        in_=pt[:, 256 * (nd - 1) : 256 * nd],
                        compare_op=GE,
                        fill=0.0,
                        base=-128,
                        channel_multiplier=-1,
                        pattern=[[1, QB]],
                    )
                else:
                    # logical diag block b: host-supplied parity mask
                    nc.gpsimd.tensor_mul(
                        out=pt[:, 256 * (nd - 1) : 256 * nd],
                        in0=pt[:, 256 * (nd - 1) : 256 * nd],
                        in1=maskb,
                    )
            return pt

        def emit_pv(t, pt, y):
            b, slot, kbs, last, first = t
            v = vA if slot == 0 else vB
            nlast = (2 * b + 1) if slot == 0 else b
            base = 256 * slot
            for j, kb in enumerate(kbs):
                for h in range(2):
                    nc.tensor.matmul(
                        y[:, base + 128 * h : base + 128 * h + HD + 1],
                        lhsT=pt[:, 256 * j + 128 * h : 256 * j + 128 * (h + 1)],
                        rhs=v[:, kb, :],
                        start=(kb == 0),
                        stop=(kb == nlast),
                        skip_group_check=True,
                    )

        def emit_finalize_a(b, y):
            """normalize y (head A) into y_sb columns for q-tiles 2b, 2b+1."""
            for h in range(2):
                r = rpool.tile([128, 1], F32, name="r", tag="r")
                with nc.allow_low_precision(reason="softmax denom recip"):
                    nc.vector.reciprocal(
                        out=r, in_=y[:, 128 * h + HD : 128 * h + HD + 1]
                    )
                nc.vector.tensor_scalar_mul(
                    out=y_sb[:, (2 * b + h) * HD : (2 * b + h + 1) * HD],
                    in0=y[:, 128 * h : 128 * h + HD],
                    scalar1=r,
                )

        def emit_finalize_b(b, y):
            """ship raw y+denom (head B partial) to DRAM."""
            yb = rpool.tile([128, 2 * (HD + 1)], BF16, name="yb", tag="yb", bufs=2)
            nc.vector.tensor_copy(
                out=yb.rearrange("p (i d) -> p i d", i=2),
                in_=y[:, 256:512].rearrange("p (i d) -> p i d", i=2)[
                    :, :, 0 : HD + 1
                ],
            )
            nc.sync.dma_start(
                out=yb_d.ap()[QB * b : QB * (b + 1), :].rearrange(
                    "(i p) d -> p i d", p=128
                ),
                in_=yb.rearrange("p (i d) -> p i d", i=2),
            )

        def emit_out_block(b):
            """transpose normalized y (head A) and project: rows 256b..+256."""
            ytp = ps_yt.tile([HD, 256], BF16, name="ytp", tag="yt")
            for h in range(2):
                nc.tensor.transpose(
                    ytp[:, 128 * h : 128 * (h + 1)],
                    y_sb[:, (2 * b + h) * HD : (2 * b + h + 1) * HD],
                    ident,
                )
            yts = rpool.tile([HD, 256], BF16, name="yts", tag="yts", bufs=2)
            nc.vector.tensor_copy(out=yts, in_=ytp)
            for h in range(2):
                po = ps_big.tile([128, 1024], F32, name="po", tag="big")
                for c0, c1 in ((0, 512), (512, 768)):
                    nc.tensor.matmul(
                        po[:, c0:c1],
                        lhsT=yts[:, 128 * h : 128 * (h + 1)],
                        rhs=wp_sb[:, c0:c1],
                        start=True,
                        stop=True,
                    )
                posb = opool.tile([128, C], BF16, name="posb", tag="po")
                nc.vector.tensor_copy(out=posb, in_=po[:, 0:C])
                r0 = QB * b + 128 * h
                nc.sync.dma_start(out=out_d.ap()[r0 : r0 + 128, :], in_=posb)

        # ---- build the task stream ----
        # per block b: A-unit (2b+2 k-blocks), B-unit (b+1 logical k-blocks)
        tasks = []  # (b, slot, kbs, is_last_group, is_first_group)

        def push_unit(b, slot, nkb):
            gs = _groups(nkb)
            for gi, kbs in enumerate(gs):
                tasks.append((b, slot, kbs, gi == len(gs) - 1, gi == 0))

        for b in range(NQB):
            push_unit(b, 0, 2 * b + 2)
            push_unit(b, 1, b + 1)

        # ---- stream emission: proj slices interleaved with attention ----
        emit_main_proj(0)
        emit_packed_proj(0)
        slices_done = 1
        packed_done = 1

        pending = None  # (task, pt, y)
        y_cur = {}  # slot -> y tile of the in-flight unit
        for ti, t in enumerate(tasks):
            b, slot, kbs, last, first = t
            # make sure data this task needs is projected
            need_s = min(NTS - 1, (2 * b + 1) // 4)
            while slices_done <= need_s:
                emit_main_proj(slices_done)
                slices_done += 1
                if slices_done % 2 == 0 and packed_done < 4:
                    emit_packed_proj(packed_done)
                    packed_done += 1
            pt = emit_scores(t)
            if first and slot == 0:
                y_cur[0] = ps_y.tile([128, 512], F32, name="y", tag="y")
            y = y_cur[0]
            if pending is not None:
                emit_pv(*pending)
                pb, pslot, _, plast, _ = pending[0]
                if plast:
                    if pslot == 0:
                        emit_finalize_a(pb, pending[2])
                    else:
                        emit_finalize_b(pb, pending[2])
                        work_q.append(lambda pb=pb: emit_out_block(pb))
            pending = (t, pt, y)
            if work_q:
                work_q.popleft()()
        emit_pv(*pending)
        pb, pslot, _, plast, _ = pending[0]
        if plast:
            if pslot == 0:
                emit_finalize_a(pb, pending[2])
            else:
                emit_finalize_b(pb, pending[2])
                work_q.append(lambda pb=pb: emit_out_block(pb))
        while slices_done < NTS:
            emit_main_proj(slices_done)
            slices_done += 1
        while packed_done < 4:
            emit_packed_proj(packed_done)
            packed_done += 1
        while work_q:
            work_q.popleft()()

    nc.compile()
    return nc


def _get_nc():
    if "nc" not in _CACHE:
        _CACHE["nc"] = _build_nc()
    return _CACHE["nc"]


def _core_inputs(x, w_attn, w_proj):
    """Per-core input dicts (bf16, host-side transpose + parity packing)."""
    import ml_dtypes

    bf16 = ml_dtypes.bfloat16
    x = np.asarray(x, dtype=np.float32).reshape(T, C)
    w_attn = np.asarray(w_attn, dtype=np.float32)
    w_proj = np.asarray(w_proj, dtype=np.float32)

    xt = np.ascontiguousarray(x.T).astype(bf16)  # [C, T]
    xt_blocks = xt.reshape(C, NKB, KB)
    # parity-packed x^T: even k-blocks (cores 0-3) / odd (cores 4-7)
    xp_even = np.ascontiguousarray(
        xt_blocks[:, 0::2, :].reshape(C, T // 2)
    ).astype(bf16)
    xp_odd = np.ascontiguousarray(
        xt_blocks[:, 1::2, :].reshape(C, T // 2)
    ).astype(bf16)

    # parity diag masks [KB, QB]: even keeps col>=p, odd keeps col>=128+p
    p = np.arange(KB)[:, None]
    col = np.arange(QB)[None, :]
    mask_even = (col >= p).astype(bf16)
    mask_odd = (col >= p + 128).astype(bf16)

    in_maps = []
    for c in range(N_CORES):
        hA = c
        hB = 8 + (c % 4)
        parity = 0 if c < 4 else 1

        def cols(w, h):
            return w[:, h * HD : (h + 1) * HD]

        wq = np.concatenate(
            [cols(w_attn[:, 0:C], hA), cols(w_attn[:, 0:C], hB)], axis=1
        ).astype(bf16)
        wk = np.concatenate(
            [cols(w_attn[:, C : 2 * C], hA), cols(w_attn[:, C : 2 * C], hB)], axis=1
        ).astype(bf16)
        wv = np.concatenate(
            [cols(w_attn[:, 2 * C : 3 * C], hA), cols(w_attn[:, 2 * C : 3 * C], hB)],
            axis=1,
        ).astype(bf16)
        wp = np.ascontiguousarray(w_proj[hA * HD : (hA + 1) * HD, :]).astype(bf16)
        in_maps.append(
            {
                "xt": xt,
                "xp": xp_even if parity == 0 else xp_odd,
                "wq": np.ascontiguousarray(wq),
                "wk": np.ascontiguousarray(wk),
                "wv": np.ascontiguousarray(wv),
                "wp": wp,
                "maskb": mask_even if parity == 0 else mask_odd,
            }
        )
    return in_maps


def _get_runner():
    """Build the shard_map'd PJRT executable once and reuse it across calls."""
    if "runner" in _CACHE:
        return _CACHE["runner"]
    import jax
    import concourse.mybir as mybir
    from concourse import bass2jax
    from jax.experimental.shard_map import shard_map
    from jax.sharding import Mesh, PartitionSpec

    nc = _get_nc()
    bass2jax.install_neuronx_cc_hook()

    in_names, out_names, out_avals, zero_outs = [], [], [], []
    for alloc in nc.m.functions[0].allocations:
        if not isinstance(alloc, mybir.MemoryLocationSet):
            continue
        name = alloc.memorylocations[0].name
        if alloc.kind == "ExternalInput":
            if nc.partition_id_tensor and name == nc.partition_id_tensor.name:
                continue
            in_names.append(name)
        elif alloc.kind == "ExternalOutput":
            shape = tuple(alloc.tensor_shape)
            dtype = mybir.dt.np(alloc.dtype)
            out_names.append(name)
            out_avals.append(jax.core.ShapedArray(shape, dtype))
            zero_outs.append(np.zeros(shape, dtype))
    n_params = len(in_names)
    all_in_names = in_names + out_names
    if nc.partition_id_tensor:
        all_in_names = all_in_names + [nc.partition_id_tensor.name]

    def _body(*args):
        operands = list(args)
        if nc.partition_id_tensor:
            operands.append(bass2jax.partition_id_tensor())
        outs = bass2jax._bass_exec_p.bind(
            *operands,
            out_avals=tuple(out_avals),
            in_names=tuple(all_in_names),
            out_names=tuple(out_names),
            lowering_input_output_aliases=(),
            sim_require_finite=True,
            sim_require_nnan=True,
            nc=nc,
        )
        return tuple(outs)

    devices = jax.devices()[:N_CORES]
    mesh = Mesh(np.asarray(devices), ("core",))
    n_out = len(out_names)
    donate = tuple(range(n_params, n_params + n_out))
    sharded = jax.jit(
        shard_map(
            _body,
            mesh=mesh,
            in_specs=(PartitionSpec("core"),) * (n_params + n_out),
            out_specs=(PartitionSpec("core"),) * n_out,
            check_rep=False,
        ),
        donate_argnums=donate,
        keep_unused=True,
    )

    def run(in_maps):
        concat_in = [
            np.concatenate([in_maps[c][name] for c in range(N_CORES)], axis=0)
            for name in in_names
        ]
        concat_zeros = [
            np.zeros((N_CORES * z.shape[0], *z.shape[1:]), z.dtype)
            for z in zero_outs
        ]
        out_arrs = sharded(*concat_in, *concat_zeros)
        return [
            {
                name: np.asarray(out_arrs[i]).reshape(
                    N_CORES, *out_avals[i].shape
                )[c]
                for i, name in enumerate(out_names)
            }
            for c in range(N_CORES)
        ]

    _CACHE["runner"] = run
    return run


def kernel(x, w_attn, w_proj):
    run = _get_runner()
    w_proj_f32 = np.asarray(w_proj, dtype=np.float32)
    in_maps = _core_inputs(np.asarray(x), np.asarray(w_attn), w_proj_f32)
    results = run(in_maps)

    out = np.zeros((T, C), dtype=np.float32)
    for c in range(N_CORES):
        out += results[c]["out"].astype(np.float32)

    # heads 8-11: combine parity partials, then project on host (fp32)
    Y = np.empty((T, 4 * HD), dtype=np.float32)
    for j in range(4):
        e = results[j]["yb"].astype(np.float32)
        o = results[4 + j]["yb"].astype(np.float32)
        num = e[:, 0:HD] + o[:, 0:HD]
        den = e[:, HD : HD + 1] + o[:, HD : HD + 1]
        Y[:, j * HD : (j + 1) * HD] = num / den
    out += Y @ w_proj_f32[8 * HD : 12 * HD, :]
    return out.reshape(1, T, C)


# revision 10
# speedup vs baseline: 1.5471x; 1.1650x over previous
"""Causal self-attention (B=1, T=4096, C=768, H=12) on 8 Trainium2 NeuronCores.

Sharding (uniform SPMD program, zero dummy work):
  - heads 0-7 live whole on cores 0-7 ("slot A", full softmax on device)
  - heads 8-11 are split between core pairs (c, c+4) by k-block PARITY:
    core c<4 handles even 128-row k-blocks, core c+4 the odd ones, each
    producing an UNNORMALIZED partial y plus the partial softmax
    denominator ("slot B"). The host adds the two partials and divides --
    exact because no max-subtraction is used (scores are small).
    The parity selection is pure data: the host packs x^T's k-blocks of
    the matching parity contiguously (xp), so both program variants are
    the same instruction stream.

Per core the device program:
  1. loads host-pretransposed x^T (bf16) -- no on-chip transposes of x
  2. projects Q^T/K^T (heads on partitions) and V in natural [t, d]
     layout (direct, no V transpose)
  3. causal attention per 256-row q-block: S^T = K^T.T Q^T per 128-wide
     k-block, exp on the scalar engine in 1536-wide batches, diagonal
     masks on gpsimd, then P^T.T V accumulated as y[q, d] with an extra
     ones-column in V giving the softmax denominator in column 64
  4. slot A: normalize y, transpose 128x64 tiles via the PE array, and
     project with this head's 64 rows of w_proj; slot B: ship raw y+denom
  The host sums the 8 partial projections and adds heads 8-11's
  contribution (a small [4096,256]x[256,768] matmul) in fp32.

All matmul inputs are bf16 (fp32 PSUM accumulation); the relative error
vs the fp32 reference stays ~1e-3, well inside the 2e-2 gate.
"""

import sys

sys.path.insert(0, "/opt/trn_rl_repo")

import numpy as np

T = 4096
C = 768
H = 12
HD = 64
N_CORES = 8
TS = 512  # t-slice for x load / projection
NTS = T // TS  # 8
QB = 256  # q-block rows
NQB = T // QB  # 16
KB = 128  # k-block rows
NKB = T // KB  # 32
GRP = 4  # k-blocks per score/exp group (4*256 = 1024 wide = 2 PSUM banks)

_CACHE = {}


def _groups(n):
    """Chunk k-block indices 0..n-1 into groups of GRP (ascending)."""
    return [list(range(g, min(g + GRP, n))) for g in range(0, n, GRP)]


def _build_nc():
    import concourse.bacc as bacc
    import concourse.tile as tile
    import concourse.mybir as mybir
    from concourse.masks import make_identity
    from contextlib import ExitStack
    import collections

    F32 = mybir.dt.float32
    BF16 = mybir.dt.bfloat16
    EXP = mybir.ActivationFunctionType.Exp
    GE = mybir.AluOpType.is_ge

    nc = bacc.Bacc(
        "TRN2",
        target_bir_lowering=False,
        debug=False,
        enable_asserts=True,
        num_devices=N_CORES,
    )
    # host-pretransposed x^T, full [C, T]
    xt_d = nc.dram_tensor("xt", [C, T], BF16, kind="ExternalInput")
    # x^T with only this core's parity of k-blocks, packed [C, T//2]
    xp_d = nc.dram_tensor("xp", [C, T // 2], BF16, kind="ExternalInput")
    wq_d = nc.dram_tensor("wq", [C, 2 * HD], BF16, kind="ExternalInput")
    wk_d = nc.dram_tensor("wk", [C, 2 * HD], BF16, kind="ExternalInput")
    wv_d = nc.dram_tensor("wv", [C, 2 * HD], BF16, kind="ExternalInput")
    wp_d = nc.dram_tensor("wp", [HD, C], BF16, kind="ExternalInput")
    mb_d = nc.dram_tensor("maskb", [KB, QB], BF16, kind="ExternalInput")
    out_d = nc.dram_tensor("out", [T, C], BF16, kind="ExternalOutput")
    yb_d = nc.dram_tensor("yb", [T, HD + 1], BF16, kind="ExternalOutput")

    scale = 1.0 / float(np.sqrt(HD))

    with ExitStack() as ctx:
        tc = ctx.enter_context(tile.TileContext(nc))
        singles = ctx.enter_context(tc.tile_pool(name="singles", bufs=1))
        ptpool = ctx.enter_context(tc.tile_pool(name="ptpool", bufs=6))
        opool = ctx.enter_context(tc.tile_pool(name="opool", bufs=2))
        rpool = ctx.enter_context(tc.tile_pool(name="rpool", bufs=4))
        ps_big = ctx.enter_context(tc.tile_pool(name="ps_big", bufs=3, space="PSUM"))
        ps_y = ctx.enter_context(tc.tile_pool(name="ps_y", bufs=2, space="PSUM"))

        # ---- persistent SBUF tensors ----
        xt = [singles.tile([128, T], BF16, name=f"xt{c}") for c in range(6)]
        xp = [singles.tile([128, T // 2], BF16, name=f"xp{c}") for c in range(6)]
        qt = singles.tile([128, T], BF16)  # rows 0:64 head A, 64:128 head B
        kt = singles.tile([128, T], BF16)  # B rows use cols 0:T//2 (packed)
        vA = singles.tile([128, NKB, HD + 1], BF16)
        vB = singles.tile([128, NKB // 2, HD + 1], BF16)
        y_sb = singles.tile([128, (T // KB) * HD], BF16)  # normalized y, head A
        wq_sb = singles.tile([128, 6, 2 * HD], BF16)
        wk_sb = singles.tile([128, 6, 2 * HD], BF16)
        wv_sb = singles.tile([128, 6, 2 * HD], BF16)
        wp_sb = singles.tile([HD, C], BF16)
        maskb = singles.tile([KB, QB], BF16)
        ident = singles.tile([128, 128], BF16)

        ident_f32 = singles.tile([128, 128], F32)
        make_identity(nc, ident_f32)
        nc.vector.tensor_copy(out=ident, in_=ident_f32)
        nc.gpsimd.memset(vA[:, :, HD : HD + 1], 1.0)
        nc.gpsimd.memset(vB[:, :, HD : HD + 1], 1.0)

        # ---- weight + mask DMAs (vector-engine queue; tiny) ----
        nc.scalar.dma_start(
            out=wq_sb, in_=wq_d.ap().rearrange("(c p) j -> p c j", p=128)
        )
        nc.scalar.dma_start(
            out=wk_sb, in_=wk_d.ap().rearrange("(c p) j -> p c j", p=128)
        )
        nc.scalar.dma_start(
            out=wv_sb, in_=wv_d.ap().rearrange("(c p) j -> p c j", p=128)
        )
        nc.scalar.dma_start(out=wp_sb, in_=wp_d.ap())
        nc.scalar.dma_start(out=maskb, in_=mb_d.ap())

        # ---- x^T input DMAs (sync-engine queue), finest slices first ----
        for c in range(6):
            nc.sync.dma_start(
                out=xt[c][:, 0:TS], in_=xt_d.ap()[128 * c : 128 * (c + 1), 0:TS]
            )
        for c in range(6):
            nc.sync.dma_start(
                out=xt[c][:, TS : 2 * TS],
                in_=xt_d.ap()[128 * c : 128 * (c + 1), TS : 2 * TS],
            )
        for c in range(6):
            nc.sync.dma_start(
                out=xp[c][:, 0:1024], in_=xp_d.ap()[128 * c : 128 * (c + 1), 0:1024]
            )
        for c in range(6):
            nc.sync.dma_start(
                out=xt[c][:, 1024:2048],
                in_=xt_d.ap()[128 * c : 128 * (c + 1), 1024:2048],
            )
        for c in range(6):
            nc.sync.dma_start(
                out=xt[c][:, 2048:3072],
                in_=xt_d.ap()[128 * c : 128 * (c + 1), 2048:3072],
            )
        for c in range(6):
            nc.sync.dma_start(
                out=xp[c][:, 1024:2048],
                in_=xp_d.ap()[128 * c : 128 * (c + 1), 1024:2048],
            )
        for c in range(6):
            nc.sync.dma_start(
                out=xt[c][:, 3072:4096],
                in_=xt_d.ap()[128 * c : 128 * (c + 1), 3072:4096],
            )

        # ---- emission helpers ----
        work_q = collections.deque()  # deferred closures to fill PE gaps

        def emit_main_proj(s):
            """q (A|B, full T), k_A, v_A for t-slice s."""
            big = ps_big.tile([128, 1024], F32, name="big", tag="big")
            c0, c1 = s * TS, (s + 1) * TS
            for c in range(6):
                nc.tensor.matmul(
                    big[:, 0:512],
                    lhsT=wq_sb[:, c, :],
                    rhs=xt[c][:, c0:c1],
                    start=(c == 0),
                    stop=(c == 5),
                )
            for c in range(6):
                nc.tensor.matmul(
                    big[0:64, 512:1024],
                    lhsT=wk_sb[:, c, 0:HD],
                    rhs=xt[c][:, c0:c1],
                    start=(c == 0),
                    stop=(c == 5),
                )
            bigv = ps_big.tile([128, 1024], F32, name="bigv", tag="big")
            for j in range(4):
                t0 = c0 + 128 * j
                for c in range(6):
                    nc.tensor.matmul(
                        bigv[:, 64 * j : 64 * (j + 1)],
                        lhsT=xt[c][:, t0 : t0 + 128],
                        rhs=wv_sb[:, c, 0:HD],
                        start=(c == 0),
                        stop=(c == 5),
                    )
            nc.vector.tensor_copy(out=qt[:, c0:c1], in_=big[:, 0:512])
            nc.vector.tensor_copy(out=kt[0:64, c0:c1], in_=big[0:64, 512:1024])
            nc.vector.tensor_copy(
                out=vA[:, 4 * s : 4 * s + 4, 0:HD],
                in_=bigv[:, 0:256].rearrange("p (j d) -> p j d", j=4),
            )

        def emit_packed_proj(p):
            """k_B, v_B for packed slice p (logical k-blocks 4p..4p+3)."""
            big = ps_big.tile([128, 1024], F32, name="bigp", tag="big")
            c0, c1 = p * TS, (p + 1) * TS
            for c in range(6):
                nc.tensor.matmul(
                    big[0:64, 0:512],
                    lhsT=wk_sb[:, c, HD : 2 * HD],
                    rhs=xp[c][:, c0:c1],
                    start=(c == 0),
                    stop=(c == 5),
                )
            for j in range(4):
                t0 = c0 + 128 * j
                for c in range(6):
                    nc.tensor.matmul(
                        big[:, 512 + 64 * j : 512 + 64 * (j + 1)],
                        lhsT=xp[c][:, t0 : t0 + 128],
                        rhs=wv_sb[:, c, HD : 2 * HD],
                        start=(c == 0),
                        stop=(c == 5),
                    )
            nc.vector.tensor_copy(out=kt[64:128, c0:c1], in_=big[0:64, 0:512])
            nc.vector.tensor_copy(
                out=vB[:, 4 * p : 4 * p + 4, 0:HD],
                in_=big[:, 512:768].rearrange("p (j d) -> p j d", j=4),
            )

        # attention tasks: one per (block b, slot, k-group)
        def emit_scores(t):
            b, slot, kbs, _, _ = t
            r0, r1 = (0, 64) if slot == 0 else (64, 128)
            gw = 256 * len(kbs)
            st = ps_big.tile([128, 1024], F32, name="st", tag="big")
            for j, kb in enumerate(kbs):
                nc.tensor.matmul(
                    st[:, 256 * j : 256 * (j + 1)],
                    lhsT=kt[r0:r1, 128 * kb : 128 * (kb + 1)],
                    rhs=qt[r0:r1, QB * b : QB * (b + 1)],
                    start=True,
                    stop=True,
                )
            pt = ptpool.tile([128, 1024], BF16, name="pt", tag="pt")
            nc.scalar.activation(out=pt[:, 0:gw], in_=st[:, 0:gw], func=EXP, scale=scale)
            if t[3]:  # last group: diagonal causal masks
                nd = len(kbs)
                if slot == 0:
                    # phys diag blocks 2b (keep q>=k: col>=p) and 2b+1
                    # (keep col >= 128+p), in place on gpsimd
                    nc.gpsimd.affine_select(
                        out=pt[:, 256 * (nd - 2) : 256 * (nd - 1)],
                        in_=pt[:, 256 * (nd - 2) : 256 * (nd - 1)],
                        compare_op=GE,
                        fill=0.0,
                        base=0,
                        channel_multiplier=-1,
                        pattern=[[1, QB]],
                    )
                    nc.gpsimd.affine_select(
                        out=pt[:, 256 * (nd - 1) : 256 * nd],
                        in_=pt[:, 256 * (nd - 1) : 256 * nd],
                        compare_op=GE,
                        fill=0.0,
                        base=-128,
                        channel_multiplier=-1,
                        pattern=[[1, QB]],
                    )
                else:
                    # logical diag block b: host-supplied parity mask
                    nc.gpsimd.tensor_mul(
                        out=pt[:, 256 * (nd - 1) : 256 * nd],
                        in0=pt[:, 256 * (nd - 1) : 256 * nd],
                        in1=maskb,
                    )
            return pt

        def emit_pv(t, pt, y):
            b, slot, kbs, last, first = t
            v = vA if slot == 0 else vB
            nlast = (2 * b + 1) if slot == 0 else b
            base = 256 * slot
            for j, kb in enumerate(kbs):
                for h in range(2):
                    nc.tensor.matmul(
                        y[:, base + 128 * h : base + 128 * h + HD + 1],
                        lhsT=pt[:, 256 * j + 128 * h : 256 * j + 128 * (h + 1)],
                        rhs=v[:, kb, :],
                        start=(kb == 0),
                        stop=(kb == nlast),
                        skip_group_check=True,
                    )

        def emit_finalize_a(b, y):
            """normalize y (head A) into y_sb columns for q-tiles 2b, 2b+1."""
            for h in range(2):
                r = rpool.tile([128, 1], F32, name="r", tag="r")
                with nc.allow_low_precision(reason="softmax denom recip"):
                    nc.vector.reciprocal(
                        out=r, in_=y[:, 128 * h + HD : 128 * h + HD + 1]
                    )
                nc.vector.tensor_scalar_mul(
                    out=y_sb[:, (2 * b + h) * HD : (2 * b + h + 1) * HD],
                    in0=y[:, 128 * h : 128 * h + HD],
                    scalar1=r,
                )

        def emit_finalize_b(b, y):
            """ship raw y+denom (head B partial) to DRAM."""
            yb = rpool.tile([128, 2 * (HD + 1)], BF16, name="yb", tag="yb", bufs=2)
            nc.vector.tensor_copy(
                out=yb.rearrange("p (i d) -> p i d", i=2),
                in_=y[:, 256:512].rearrange("p (i d) -> p i d", i=2)[
                    :, :, 0 : HD + 1
                ],
            )
            nc.sync.dma_start(
                out=yb_d.ap()[QB * b : QB * (b + 1), :].rearrange(
                    "(i p) d -> p i d", p=128
                ),
                in_=yb.rearrange("p (i d) -> p i d", i=2),
            )

        def emit_out_block(b):
            """transpose normalized y (head A) and project: rows 256b..+256."""
            ytp = ps_big.tile([HD, 256], BF16, name="ytp", tag="big")
            for h in range(2):
                nc.tensor.transpose(
                    ytp[:, 128 * h : 128 * (h + 1)],
                    y_sb[:, (2 * b + h) * HD : (2 * b + h + 1) * HD],
                    ident,
                )
            yts = rpool.tile([HD, 256], BF16, name="yts", tag="yts", bufs=2)
            nc.vector.tensor_copy(out=yts, in_=ytp)
            for h in range(2):
                po = ps_big.tile([128, 1024], F32, name="po", tag="big")
                for c0, c1 in ((0, 512), (512, 768)):
                    nc.tensor.matmul(
                        po[:, c0:c1],
                        lhsT=yts[:, 128 * h : 128 * (h + 1)],
                        rhs=wp_sb[:, c0:c1],
                        start=True,
                        stop=True,
                    )
                posb = opool.tile([128, C], BF16, name="posb", tag="po")
                nc.vector.tensor_copy(out=posb, in_=po[:, 0:C])
                r0 = QB * b + 128 * h
                nc.sync.dma_start(out=out_d.ap()[r0 : r0 + 128, :], in_=posb)

        # ---- build the task stream ----
        # per block b: A-unit (2b+2 k-blocks), B-unit (b+1 logical k-blocks)
        tasks = []  # (b, slot, kbs, is_last_group, is_first_group)

        def push_unit(b, slot, nkb):
            gs = _groups(nkb)
            for gi, kbs in enumerate(gs):
                tasks.append((b, slot, kbs, gi == len(gs) - 1, gi == 0))

        for b in range(NQB):
            push_unit(b, 0, 2 * b + 2)
            push_unit(b, 1, b + 1)

        # ---- stream emission: proj slices interleaved with attention ----
        emit_main_proj(0)
        emit_packed_proj(0)
        slices_done = 1
        packed_done = 1

        LAG = 2
        pending = collections.deque()  # (task, pt, y), PV emitted LAG tasks later
        y_cur = {}

        def retire(p):
            emit_pv(*p)
            pb, pslot, _, plast, _ = p[0]
            if plast:
                if pslot == 0:
                    emit_finalize_a(pb, p[2])
                else:
                    emit_finalize_b(pb, p[2])
                    work_q.append(lambda pb=pb: emit_out_block(pb))

        for ti, t in enumerate(tasks):
            b, slot, kbs, last, first = t
            # make sure data this task needs is projected
            need_s = min(NTS - 1, (2 * b + 1) // 4)
            while slices_done <= need_s:
                emit_main_proj(slices_done)
                slices_done += 1
                if slices_done % 2 == 0 and packed_done < 4:
                    emit_packed_proj(packed_done)
                    packed_done += 1
            pt = emit_scores(t)
            if first and slot == 0:
                y_cur[0] = ps_y.tile([128, 512], F32, name="y", tag="y")
            y = y_cur[0]
            pending.append((t, pt, y))
            if len(pending) > LAG:
                retire(pending.popleft())
            if work_q:
                work_q.popleft()()
        while pending:
            retire(pending.popleft())
        while slices_done < NTS:
            emit_main_proj(slices_done)
            slices_done += 1
        while packed_done < 4:
            emit_packed_proj(packed_done)
            packed_done += 1
        while work_q:
            work_q.popleft()()

    nc.compile()
    return nc


def _get_nc():
    if "nc" not in _CACHE:
        _CACHE["nc"] = _build_nc()
    return _CACHE["nc"]


def _core_inputs(x, w_attn, w_proj):
    """Per-core input dicts (bf16, host-side transpose + parity packing)."""
    import ml_dtypes

    bf16 = ml_dtypes.bfloat16
    x = np.asarray(x, dtype=np.float32).reshape(T, C)
    w_attn = np.asarray(w_attn, dtype=np.float32)
    w_proj = np.asarray(w_proj, dtype=np.float32)

    xt = np.ascontiguousarray(x.T).astype(bf16)  # [C, T]
    xt_blocks = xt.reshape(C, NKB, KB)
    # parity-packed x^T: even k-blocks (cores 0-3) / odd (cores 4-7)
    xp_even = np.ascontiguousarray(
        xt_blocks[:, 0::2, :].reshape(C, T // 2)
    ).astype(bf16)
    xp_odd = np.ascontiguousarray(
        xt_blocks[:, 1::2, :].reshape(C, T // 2)
    ).astype(bf16)

    # parity diag masks [KB, QB]: even keeps col>=p, odd keeps col>=128+p
    p = np.arange(KB)[:, None]
    col = np.arange(QB)[None, :]
    mask_even = (col >= p).astype(bf16)
    mask_odd = (col >= p + 128).astype(bf16)

    in_maps = []
    for c in range(N_CORES):
        hA = c
        hB = 8 + (c % 4)
        parity = 0 if c < 4 else 1

        def cols(w, h):
            return w[:, h * HD : (h + 1) * HD]

        wq = np.concatenate(
            [cols(w_attn[:, 0:C], hA), cols(w_attn[:, 0:C], hB)], axis=1
        ).astype(bf16)
        wk = np.concatenate(
            [cols(w_attn[:, C : 2 * C], hA), cols(w_attn[:, C : 2 * C], hB)], axis=1
        ).astype(bf16)
        wv = np.concatenate(
            [cols(w_attn[:, 2 * C : 3 * C], hA), cols(w_attn[:, 2 * C : 3 * C], hB)],
            axis=1,
        ).astype(bf16)
        wp = np.ascontiguousarray(w_proj[hA * HD : (hA + 1) * HD, :]).astype(bf16)
        in_maps.append(
            {
                "xt": xt,
                "xp": xp_even if parity == 0 else xp_odd,
                "wq": np.ascontiguousarray(wq),
                "wk": np.ascontiguousarray(wk),
                "wv": np.ascontiguousarray(wv),
                "wp": wp,
                "maskb": mask_even if parity == 0 else mask_odd,
            }
        )
    return in_maps


def _get_runner():
    """Build the shard_map'd PJRT executable once and reuse it across calls."""
    if "runner" in _CACHE:
        return _CACHE["runner"]
    import jax
    import concourse.mybir as mybir
    from concourse import bass2jax
    from jax.experimental.shard_map import shard_map
    from jax.sharding import Mesh, PartitionSpec

    nc = _get_nc()
    bass2jax.install_neuronx_cc_hook()

    in_names, out_names, out_avals, zero_outs = [], [], [], []
    for alloc in nc.m.functions[0].allocations:
        if not isinstance(alloc, mybir.MemoryLocationSet):
            continue
        name = alloc.memorylocations[0].name
        if alloc.kind == "ExternalInput":
            if nc.partition_id_tensor and name == nc.partition_id_tensor.name:
                continue
            in_names.append(name)
        elif alloc.kind == "ExternalOutput":
            shape = tuple(alloc.tensor_shape)
            dtype = mybir.dt.np(alloc.dtype)
            out_names.append(name)
            out_avals.append(jax.core.ShapedArray(shape, dtype))
            zero_outs.append(np.zeros(shape, dtype))
    n_params = len(in_names)
    all_in_names = in_names + out_names
    if nc.partition_id_tensor:
        all_in_names = all_in_names + [nc.partition_id_tensor.name]

    def _body(*args):
        operands = list(args)
        if nc.partition_id_tensor:
            operands.append(bass2jax.partition_id_tensor())
        outs = bass2jax._bass_exec_p.bind(
            *operands,
            out_avals=tuple(out_avals),
            in_names=tuple(all_in_names),
            out_names=tuple(out_names),
            lowering_input_output_aliases=(),
            sim_require_finite=True,
            sim_require_nnan=True,
            nc=nc,
        )
        return tuple(outs)

    devices = jax.devices()[:N_CORES]
    mesh = Mesh(np.asarray(devices), ("core",))
    n_out = len(out_names)
    donate = tuple(range(n_params, n_params + n_out))
    sharded = jax.jit(
        shard_map(
            _body,
            mesh=mesh,
            in_specs=(PartitionSpec("core"),) * (n_params + n_out),
            out_specs=(PartitionSpec("core"),) * n_out,
            check_rep=False,
        ),
        donate_argnums=donate,
        keep_unused=True,
    )

    def run(in_maps):
        concat_in = [
            np.concatenate([in_maps[c][name] for c in range(N_CORES)], axis=0)
            for name in in_names
        ]
        concat_zeros = [
            np.zeros((N_CORES * z.shape[0], *z.shape[1:]), z.dtype)
            for z in zero_outs
        ]
        out_arrs = sharded(*concat_in, *concat_zeros)
        return [
            {
                name: np.asarray(out_arrs[i]).reshape(
                    N_CORES, *out_avals[i].shape
                )[c]
                for i, name in enumerate(out_names)
            }
            for c in range(N_CORES)
        ]

    _CACHE["runner"] = run
    return run


def kernel(x, w_attn, w_proj):
    run = _get_runner()
    w_proj_f32 = np.asarray(w_proj, dtype=np.float32)
    in_maps = _core_inputs(np.asarray(x), np.asarray(w_attn), w_proj_f32)
    results = run(in_maps)

    out = np.zeros((T, C), dtype=np.float32)
    for c in range(N_CORES):
        out += results[c]["out"].astype(np.float32)

    # heads 8-11: combine parity partials, then project on host (fp32)
    Y = np.empty((T, 4 * HD), dtype=np.float32)
    for j in range(4):
        e = results[j]["yb"].astype(np.float32)
        o = results[4 + j]["yb"].astype(np.float32)
        num = e[:, 0:HD] + o[:, 0:HD]
        den = e[:, HD : HD + 1] + o[:, HD : HD + 1]
        Y[:, j * HD : (j + 1) * HD] = num / den
    out += Y @ w_proj_f32[8 * HD : 12 * HD, :]
    return out.reshape(1, T, C)
